# revision 21
# baseline (speedup 1.0000x reference)
"""VQ codebook lookup (nn_MetaSlot_20890720928570) on 8 TRN2 NeuronCores.

Reference computation (per token row e, codebook rows t):
    dist = (e_sq - 2 e@t.T) + t_sq        (fp32, rounded exactly this way)
    zidx = argmax(softmax(-dist)) = lowest-index argmin of dist
    quant = templat[zidx]

Numerics: dist ~ 512, so fp32 rounds it to a ~6e-5 grid; exact index ties are
common (85/16384 rows) and are resolved by lowest index. Matching the
reference bit-for-bit requires x = e@t.T accurate to ~1e-7 absolute and the
same two-step rounding of (e_sq - 2x) + t_sq. e_sq itself only needs to be
*some* fp32 value: any same-binade fp32 e_sq error shifts all candidates by
an exact multiple of the dist grid, preserving every tie and rounding.

Implementation per core (2048 tokens, tokens on SBUF partitions):
  - operands transposed on the PE (fp32 transpose-mode matmul), the 2^12
    templat scale applied by the exact power-of-2 ACT copy out of PSUM, and
    the hi/lo fp16 split derived from the SBUF copy. DMA-xbar transpose is
    ~25 GB/s serialized on the sync sequencer -- far too slow here.
  - x via 3 fp16 matmul passes (e_hi@t_hi + e_lo@t_hi + e_hi@t_lo); with the
    2^12 pre-scale the PSUM result is 2^11*(2x) exactly. fp16 products are
    exact in fp32 PSUM; total |err| ~ 4e-8 -> zero index flips vs fp32.
  - epilogue in the 2^11-scaled domain (power-of-2 scaling commutes with fp32
    rounding exactly): w = (X + (-2^11 e_sq)) + (-2^11 t_sq) via one
    scalar_tensor_tensor per 512-code chunk; argmax w == argmin dist.
  - vector.max + max_index give the argmax with lowest-index tie-break,
    matching jnp.argmax.  quant gathered from DRAM templat by indirect DMA.

Sharding: data-parallel on the token axis, 2048 tokens per core, codebook
replicated; outputs concatenated on host.
"""
import numpy as np

import concourse.bass as bass
import concourse.bacc as bacc
import concourse.mybir as mybir
import concourse.tile as tile
from concourse import bass_utils
from concourse.masks import make_identity

P = 128            # partitions
D = 512            # embed dim
NCODE = 2048       # codebook entries
TOK_PER_CORE = 2048
N_CORES = 8
KC = D // P        # 4 contraction chunks
MT = TOK_PER_CORE // P   # 16 token tiles per core
NCH = NCODE // 512       # 4 code chunks of 512
TSCALE = 2048.0    # 2^11: scaled comparison domain (w = -2^11 * dist + const)
TSPLIT = 4096.0    # 2^12: templat pre-scale, so PSUM X = 2^12 x = 2^11 * (2x)

f32 = mybir.dt.float32
f16 = mybir.dt.float16
u32 = mybir.dt.uint32


def _build_nc():
    nc = bacc.Bacc("TRN2", debug=False)
    enc_d = nc.dram_tensor("encode", [TOK_PER_CORE, D], f32, kind="ExternalInput").ap()
    tem_d = nc.dram_tensor("templat", [NCODE, D], f32, kind="ExternalInput").ap()
    quant_d = nc.dram_tensor("quant", [TOK_PER_CORE, D], f32, kind="ExternalOutput").ap()
    zidx_d = nc.dram_tensor("zidx", [TOK_PER_CORE, 1], u32, kind="ExternalOutput").ap()

    with tile.TileContext(nc) as tc:
        with tc.tile_pool(name="persist", bufs=1) as persist, \
             tc.tile_pool(name="tprep", bufs=3) as tprep, \
             tc.tile_pool(name="eprep", bufs=3) as eprep, \
             tc.tile_pool(name="etrans", bufs=3) as etrans, \
             tc.tile_pool(name="epi", bufs=3) as epi, \
             tc.tile_pool(name="out", bufs=3) as outp, \
             tc.tile_pool(name="psum", bufs=1, space="PSUM") as psum, \
             tc.tile_pool(name="psum_t", bufs=3, space="PSUM") as psum_t:

            # ---- persistent tiles ----
            thT_all = persist.tile([P, KC * NCODE], f16, tag="thT_all")
            tlT_all = persist.tile([P, KC * NCODE], f16, tag="tlT_all")
            thT = [thT_all[:, k * NCODE:(k + 1) * NCODE] for k in range(KC)]
            tlT = [tlT_all[:, k * NCODE:(k + 1) * NCODE] for k in range(KC)]
            neg_tsq = persist.tile([P, NCODE], f32, tag="neg_tsq")  # -2^11*t_sq
            mones = persist.tile([P, P], f32, tag="mones")
            nc.vector.memset(mones[:], -1.0)
            ident1 = persist.tile([P, P], f32, tag="ident1")  # I
            make_identity(nc, ident1[:])

            # squares of the scaled transposed templat, for the t_sq matmul
            sqT_all = persist.tile([P, KC * NCODE], f32, tag="sqT_all")
            sqT = [sqT_all[:, k * NCODE:(k + 1) * NCODE] for k in range(KC)]

            # ---- stage A: templat prep (PE transpose + split from PSUM) ----
            for ct in range(NCODE // P):
                tf = tprep.tile([P, D], f32, tag="tf")
                nc.sync.dma_start(out=tf[:], in_=tem_d[ct * P:(ct + 1) * P, :])
                pst = psum_t.tile([P, D], f32, tag="trp")  # holds 2^12 * t.T chunks
                t32 = tprep.tile([P, D], f32, tag="t32")
                for k in range(KC):
                    sl = slice(k * P, (k + 1) * P)
                    # transpose mode needs a permutation rhs; fold the 2^12
                    # scale into the (exact) power-of-2 ACT copy instead
                    nc.tensor.transpose(pst[:, sl], tf[:, sl], ident1[:])
                cs = slice(ct * P, (ct + 1) * P)
                thv = thT_all[:].rearrange("p (k c) -> p k c", k=KC)[:, :, cs]
                tlv = tlT_all[:].rearrange("p (k c) -> p k c", k=KC)[:, :, cs]
                sqv = sqT_all[:].rearrange("p (k c) -> p k c", k=KC)[:, :, cs]
                t32v = t32[:].rearrange("p (k c) -> p k c", k=KC)
                nc.scalar.mul(t32[:], pst[:], TSPLIT)
                nc.vector.tensor_copy(out=thv, in_=t32v)
                nc.vector.tensor_sub(tlv, t32v, thv)
                nc.gpsimd.tensor_mul(sqv, t32v, t32v)

            # neg_tsq = -(2^11 t_sq) broadcast on partitions via minus-ones
            # matmul: psum accumulates -sum((2^12 t)^2) = -2^24 t_sq.
            s01 = persist.tile([P, NCODE], f32, tag="s01")
            s23 = persist.tile([P, NCODE], f32, tag="s23")
            sqsum = persist.tile([P, NCODE], f32, tag="sqsum")
            nc.vector.tensor_add(s01[:], sqT[0], sqT[1])
            nc.gpsimd.tensor_add(s23[:], sqT[2], sqT[3])
            nc.vector.tensor_add(sqsum[:], s01[:], s23[:])
            for n in range(NCH):
                ps_t = psum.tile([P, 512], f32, tag=f"X{n}", name=f"pst{n}")
                nc.tensor.matmul(
                    ps_t[:], lhsT=mones[:], rhs=sqsum[:, n * 512:(n + 1) * 512],
                    start=True, stop=True)
                nc.scalar.mul(neg_tsq[:, n * 512:(n + 1) * 512], ps_t[:],
                              TSCALE / (TSPLIT * TSPLIT))

            # ---- stage B: token tiles ----
            for mt in range(MT):
                ef = eprep.tile([P, D], f32, tag="ef")
                nc.sync.dma_start(out=ef[:], in_=enc_d[mt * P:(mt + 1) * P, :])
                scr = eprep.tile([P, D], f32, tag="scr")
                nesq = eprep.tile([P, 1], f32, tag="nesq")
                # nesq = -2^11 * e_sq; e_sq tolerates any same-binade fp32
                # value (see docstring), so ACT Square + accum reduction is fine
                nc.scalar.activation(out=scr[:], in_=ef[:],
                                     func=mybir.ActivationFunctionType.Square,
                                     accum_out=nesq[:])
                nc.vector.tensor_scalar_mul(nesq[:], nesq[:], -TSCALE)

                pse = psum_t.tile([P, D], f32, tag="trp", name="pse")  # e.T chunks
                ehT_t = etrans.tile([P, D], f16, tag="ehT")
                elT_t = etrans.tile([P, D], f16, tag="elT")
                for k in range(KC):
                    sl = slice(k * P, (k + 1) * P)
                    nc.tensor.transpose(pse[:, sl], ef[:, sl], ident1[:])
                nc.scalar.copy(ehT_t[:], pse[:])
                nc.vector.tensor_sub(elT_t[:], pse[:], ehT_t[:])
                ehT = [ehT_t[:, k * P:(k + 1) * P] for k in range(KC)]
                elT = [elT_t[:, k * P:(k + 1) * P] for k in range(KC)]

                # 12 accumulation steps x 4 psum banks; lhsT reused across banks
                X = [psum.tile([P, 512], f32, tag=f"X{n}", name=f"Xp{n}") for n in range(NCH)]
                si = 0
                for aT, bT in [(ehT, thT), (elT, thT), (ehT, tlT)]:
                    for k in range(KC):
                        for n in range(NCH):
                            nc.tensor.matmul(
                                X[n][:], lhsT=aT[k][:],
                                rhs=bT[k][:, n * 512:(n + 1) * 512],
                                start=(si == 0), stop=(si == 11))
                        si += 1

                # w = (X + (-2^11 e_sq)) + (-2^11 t_sq)  == -2^11 * dist + const
                w = epi.tile([P, NCODE], f32, tag="w")
                for n in range(NCH):
                    nc.vector.scalar_tensor_tensor(
                        out=w[:, n * 512:(n + 1) * 512], in0=X[n][:],
                        scalar=nesq[:, :1], in1=neg_tsq[:, n * 512:(n + 1) * 512],
                        op0=mybir.AluOpType.add, op1=mybir.AluOpType.add)

                mx8 = outp.tile([P, 8], f32, tag="mx8")
                mi8 = outp.tile([P, 8], u32, tag="mi8")
                nc.vector.max(out=mx8[:], in_=w[:])
                nc.vector.max_index(out=mi8[:], in_max=mx8[:], in_values=w[:])

                q = outp.tile([P, D], f32, tag="q")
                nc.gpsimd.indirect_dma_start(
                    out=q[:], out_offset=None, in_=tem_d[:],
                    in_offset=bass.IndirectOffsetOnAxis(ap=mi8[:, :1], axis=0))
                nc.sync.dma_start(out=quant_d[mt * P:(mt + 1) * P, :], in_=q[:])
                nc.sync.dma_start(out=zidx_d[mt * P:(mt + 1) * P, :], in_=mi8[:, 0:1])

    nc.compile()
    return nc


_NC = None


def _get_nc():
    global _NC
    if _NC is None:
        _NC = _build_nc()
    return _NC


def kernel(encode: np.ndarray, templat: np.ndarray):
    encode = np.ascontiguousarray(encode, dtype=np.float32)
    templat = np.ascontiguousarray(templat, dtype=np.float32)
    nc = _get_nc()
    in_maps = [
        {"encode": encode[c * TOK_PER_CORE:(c + 1) * TOK_PER_CORE],
         "templat": templat}
        for c in range(N_CORES)
    ]
    res = bass_utils.run_bass_kernel_spmd(nc, in_maps, core_ids=list(range(N_CORES)))
    quant = np.concatenate([res.results[c]["quant"] for c in range(N_CORES)], axis=0)
    zidx = np.concatenate(
        [res.results[c]["zidx"].reshape(-1) for c in range(N_CORES)], axis=0)
    return quant, zidx.astype(np.int32)


# revision 22
# speedup vs baseline: 1.1162x; 1.1162x over previous
"""VQ codebook lookup (nn_MetaSlot_20890720928570) on 8 TRN2 NeuronCores.

Reference computation (per token row e, codebook rows t):
    dist = (e_sq - 2 e@t.T) + t_sq        (fp32, rounded exactly this way)
    zidx = argmax(softmax(-dist)) = lowest-index argmin of dist
    quant = templat[zidx]

Numerics: dist ~ 512, so fp32 rounds it to a ~6e-5 grid; exact index ties are
common (85/16384 rows) and are resolved by lowest index. Matching the
reference bit-for-bit requires x = e@t.T accurate to ~1e-7 absolute and the
same two-step rounding of (e_sq - 2x) + t_sq. e_sq itself only needs to be
*some* fp32 value: any same-binade fp32 e_sq error shifts all candidates by
an exact multiple of the dist grid, preserving every tie and rounding.

Implementation per core (2048 tokens, tokens on SBUF partitions):
  - operands transposed on the PE (fp32 transpose-mode matmul), the 2^12
    templat scale applied by the exact power-of-2 ACT copy out of PSUM, and
    the hi/lo fp16 split derived from the SBUF copy. DMA-xbar transpose is
    ~25 GB/s serialized on the sync sequencer -- far too slow here.
  - x via 3 fp16 matmul passes (e_hi@t_hi + e_lo@t_hi + e_hi@t_lo); with the
    2^12 pre-scale the PSUM result is 2^11*(2x) exactly. fp16 products are
    exact in fp32 PSUM; total |err| ~ 4e-8 -> zero index flips vs fp32.
  - epilogue in the 2^11-scaled domain (power-of-2 scaling commutes with fp32
    rounding exactly): w = (X + (-2^11 e_sq)) + (-2^11 t_sq) via one
    scalar_tensor_tensor per 512-code chunk; argmax w == argmin dist.
  - vector.max + max_index give the argmax with lowest-index tie-break,
    matching jnp.argmax.  quant gathered from DRAM templat by indirect DMA.

Sharding: data-parallel on the token axis, 2048 tokens per core, codebook
replicated; outputs concatenated on host.
"""
import numpy as np

import concourse.bass as bass
import concourse.bacc as bacc
import concourse.mybir as mybir
import concourse.tile as tile
from concourse import bass_utils
from concourse.masks import make_identity

P = 128            # partitions
D = 512            # embed dim
NCODE = 2048       # codebook entries
TOK_PER_CORE = 2048
N_CORES = 8
KC = D // P        # 4 contraction chunks
MT = TOK_PER_CORE // P   # 16 token tiles per core
NCH = NCODE // 512       # 4 code chunks of 512
TSCALE = 2048.0    # 2^11: scaled comparison domain (w = -2^11 * dist + const)
TSPLIT = 4096.0    # 2^12: templat pre-scale, so PSUM X = 2^12 x = 2^11 * (2x)

f32 = mybir.dt.float32
f16 = mybir.dt.float16
u32 = mybir.dt.uint32


def _build_nc():
    nc = bacc.Bacc("TRN2", debug=False)
    enc_d = nc.dram_tensor("encode", [TOK_PER_CORE, D], f32, kind="ExternalInput").ap()
    tem_d = nc.dram_tensor("templat", [NCODE, D], f32, kind="ExternalInput").ap()
    quant_d = nc.dram_tensor("quant", [TOK_PER_CORE, D], f32, kind="ExternalOutput").ap()
    zidx_d = nc.dram_tensor("zidx", [TOK_PER_CORE, 1], u32, kind="ExternalOutput").ap()

    with tile.TileContext(nc) as tc:
        with tc.tile_pool(name="persist", bufs=1) as persist, \
             tc.tile_pool(name="tprep", bufs=3) as tprep, \
             tc.tile_pool(name="eprep", bufs=3) as eprep, \
             tc.tile_pool(name="etrans", bufs=3) as etrans, \
             tc.tile_pool(name="epi", bufs=3) as epi, \
             tc.tile_pool(name="out", bufs=3) as outp, \
             tc.tile_pool(name="psum", bufs=1, space="PSUM") as psum, \
             tc.tile_pool(name="psum_t", bufs=3, space="PSUM") as psum_t:

            # ---- persistent tiles ----
            thT = [persist.tile([P, NCODE], f16, tag=f"thT{k}", name=f"thT{k}") for k in range(KC)]
            tlT = [persist.tile([P, NCODE], f16, tag=f"tlT{k}", name=f"tlT{k}") for k in range(KC)]
            neg_tsq = persist.tile([P, NCODE], f32, tag="neg_tsq")  # -2^11*t_sq
            mones = persist.tile([P, P], f32, tag="mones")
            nc.vector.memset(mones[:], -1.0)
            ident1 = persist.tile([P, P], f32, tag="ident1")  # I
            make_identity(nc, ident1[:])

            # squares of the scaled transposed templat, for the t_sq matmul
            sqT = [persist.tile([P, NCODE], f32, tag=f"sqT{k}", name=f"sqT{k}") for k in range(KC)]

            # ---- stage A: templat prep (PE transpose + split from PSUM) ----
            for ct in range(NCODE // P):
                tf = tprep.tile([P, D], f32, tag="tf")
                nc.sync.dma_start(out=tf[:], in_=tem_d[ct * P:(ct + 1) * P, :])
                pst = psum_t.tile([P, D], f32, tag="trp")  # holds 2^12 * t.T chunks
                t32 = tprep.tile([P, D], f32, tag="t32")
                for k in range(KC):
                    sl = slice(k * P, (k + 1) * P)
                    cs = slice(ct * P, (ct + 1) * P)
                    # transpose mode needs a permutation rhs; fold the 2^12
                    # scale into the (exact) power-of-2 ACT copy instead
                    nc.tensor.transpose(pst[:, sl], tf[:, sl], ident1[:])
                    nc.scalar.mul(t32[:, sl], pst[:, sl], TSPLIT)
                    nc.vector.tensor_copy(out=thT[k][:, cs], in_=t32[:, sl])
                    nc.vector.tensor_sub(tlT[k][:, cs], t32[:, sl], thT[k][:, cs])
                    nc.gpsimd.tensor_mul(sqT[k][:, cs], t32[:, sl], t32[:, sl])

            # neg_tsq = -(2^11 t_sq) broadcast on partitions via minus-ones
            # matmul: psum accumulates -sum((2^12 t)^2) = -2^24 t_sq.
            s01 = persist.tile([P, NCODE], f32, tag="s01")
            s23 = persist.tile([P, NCODE], f32, tag="s23")
            sqsum = persist.tile([P, NCODE], f32, tag="sqsum")
            nc.vector.tensor_add(s01[:], sqT[0][:], sqT[1][:])
            nc.gpsimd.tensor_add(s23[:], sqT[2][:], sqT[3][:])
            nc.vector.tensor_add(sqsum[:], s01[:], s23[:])
            for n in range(NCH):
                ps_t = psum.tile([P, 512], f32, tag=f"X{n}", name=f"pst{n}")
                nc.tensor.matmul(
                    ps_t[:], lhsT=mones[:], rhs=sqsum[:, n * 512:(n + 1) * 512],
                    start=True, stop=True)
                nc.scalar.mul(neg_tsq[:, n * 512:(n + 1) * 512], ps_t[:],
                              TSCALE / (TSPLIT * TSPLIT))

            # ---- stage B: token tiles ----
            for mt in range(MT):
                ef = eprep.tile([P, D], f32, tag="ef")
                nc.sync.dma_start(out=ef[:], in_=enc_d[mt * P:(mt + 1) * P, :])
                scr = eprep.tile([P, D], f32, tag="scr")
                nesq = eprep.tile([P, 1], f32, tag="nesq")
                # nesq = -2^11 * e_sq; e_sq tolerates any same-binade fp32
                # value (see docstring), so ACT Square + accum reduction is fine
                nc.scalar.activation(out=scr[:], in_=ef[:],
                                     func=mybir.ActivationFunctionType.Square,
                                     accum_out=nesq[:])
                nc.vector.tensor_scalar_mul(nesq[:], nesq[:], -TSCALE)

                pse = psum_t.tile([P, D], f32, tag="trp", name="pse")  # e.T chunks
                ehT = [etrans.tile([P, P], f16, tag=f"ehT{k}", name=f"ehT{k}") for k in range(KC)]
                elT = [etrans.tile([P, P], f16, tag=f"elT{k}", name=f"elT{k}") for k in range(KC)]
                for k in range(KC):
                    sl = slice(k * P, (k + 1) * P)
                    nc.tensor.transpose(pse[:, sl], ef[:, sl], ident1[:])
                    nc.scalar.copy(ehT[k][:], pse[:, sl])
                    nc.vector.tensor_sub(elT[k][:], pse[:, sl], ehT[k][:])

                # 12 accumulation steps x 4 psum banks; lhsT reused across banks
                X = [psum.tile([P, 512], f32, tag=f"X{n}", name=f"Xp{n}") for n in range(NCH)]
                si = 0
                for aT, bT in [(ehT, thT), (elT, thT), (ehT, tlT)]:
                    for k in range(KC):
                        for n in range(NCH):
                            nc.tensor.matmul(
                                X[n][:], lhsT=aT[k][:],
                                rhs=bT[k][:, n * 512:(n + 1) * 512],
                                start=(si == 0), stop=(si == 11))
                        si += 1

                # w = (X + (-2^11 e_sq)) + (-2^11 t_sq)  == -2^11 * dist + const
                w = epi.tile([P, NCODE], f32, tag="w")
                for n in range(NCH):
                    nc.vector.scalar_tensor_tensor(
                        out=w[:, n * 512:(n + 1) * 512], in0=X[n][:],
                        scalar=nesq[:, :1], in1=neg_tsq[:, n * 512:(n + 1) * 512],
                        op0=mybir.AluOpType.add, op1=mybir.AluOpType.add)

                mx8 = outp.tile([P, 8], f32, tag="mx8")
                mi8 = outp.tile([P, 8], u32, tag="mi8")
                nc.vector.max(out=mx8[:], in_=w[:])
                nc.vector.max_index(out=mi8[:], in_max=mx8[:], in_values=w[:])

                q = outp.tile([P, D], f32, tag="q")
                nc.gpsimd.indirect_dma_start(
                    out=q[:], out_offset=None, in_=tem_d[:],
                    in_offset=bass.IndirectOffsetOnAxis(ap=mi8[:, :1], axis=0))
                nc.sync.dma_start(out=quant_d[mt * P:(mt + 1) * P, :], in_=q[:])
                nc.sync.dma_start(out=zidx_d[mt * P:(mt + 1) * P, :], in_=mi8[:, 0:1])

    nc.compile()
    return nc


_NC = None


def _get_nc():
    global _NC
    if _NC is None:
        _NC = _build_nc()
    return _NC


def kernel(encode: np.ndarray, templat: np.ndarray):
    encode = np.ascontiguousarray(encode, dtype=np.float32)
    templat = np.ascontiguousarray(templat, dtype=np.float32)
    nc = _get_nc()
    in_maps = [
        {"encode": encode[c * TOK_PER_CORE:(c + 1) * TOK_PER_CORE],
         "templat": templat}
        for c in range(N_CORES)
    ]
    res = bass_utils.run_bass_kernel_spmd(nc, in_maps, core_ids=list(range(N_CORES)))
    quant = np.concatenate([res.results[c]["quant"] for c in range(N_CORES)], axis=0)
    zidx = np.concatenate(
        [res.results[c]["zidx"].reshape(-1) for c in range(N_CORES)], axis=0)
    return quant, zidx.astype(np.int32)
